# revision 12
# baseline (speedup 1.0000x reference)
"""Single-step LSTM cell (B=131072, E=H=128) on 8 Trainium2 NeuronCores.

Strategy: pure data-parallel over the batch. Each core handles 16384 rows.
Host-side we pre-transpose each shard (x^T, h^T, c^T: [128, Bc] bf16) so the
contraction dim (E/H) lands on SBUF partitions — no on-chip transposes.

Per 2048-column group the four gate pre-activations are computed as
per-gate PSUM tiles [128, 2048] (4 banks each, 2-slot ring) with two
bf16 FD=1024 matmuls per stationary (W@x, U@h accumulate). The gate
bias enters through the ACT engine's per-partition bias operand on the
activation instruction itself — no bias matmuls at all. Each gate gets
one ACTIVATE (Sigmoid for i/f/o, Tanh for c~) at FD=2048, and tanh(c)
is one more ACTIVATE from SBUF. Elementwise (f*c_prev + i*c~, o*tanh(c))
runs on DVE fully in bf16 (2x mode). The c path moves as bf16 in both
directions, halving its HBM traffic and keeping every DVE op in 2x.

Steady state is ACT-bound: 5 ACTIVATEs x (2048+352)cyc / 1.2GHz = 10us
per group, 8 groups = 80us/core, with PE (~55us), DVE (~36us) and DMA
(20 MiB ~ 59us) all hidden underneath.
"""

import numpy as np

B, E, H = 131072, 128, 128
NCORES = 8
BC = B // NCORES        # 16384 batch rows per core
GROUP = 2048            # batch cols per pipeline group
HALF = GROUP // 2       # matmul moving-operand FD (bf16 max 1024)
NG = BC // GROUP

_CACHE = {}

# gate order in the weight/bias concatenation AND in per-group compute:
# f first (unblocks m1 = f*c_prev early), then c~, then i, then o.
GF, GC, GI, GO = 0, 1, 2, 3


def _build_nc():
    import concourse.bacc as bacc
    import concourse.mybir as mybir
    import concourse.tile as tile

    f32 = mybir.dt.float32
    bf = mybir.dt.bfloat16
    AF = mybir.ActivationFunctionType

    nc = bacc.Bacc("TRN2", target_bir_lowering=False, debug=False,
                   num_devices=NCORES)

    xT = nc.dram_tensor("xT", [E, BC], bf, kind="ExternalInput").ap()
    hT = nc.dram_tensor("hT", [H, BC], bf, kind="ExternalInput").ap()
    cT = nc.dram_tensor("cT", [H, BC], bf, kind="ExternalInput").ap()
    W = nc.dram_tensor("W", [E, 4 * H], bf, kind="ExternalInput").ap()
    U = nc.dram_tensor("U", [H, 4 * H], bf, kind="ExternalInput").ap()
    bias = nc.dram_tensor("b", [H, 4], f32, kind="ExternalInput").ap()
    hT_out = nc.dram_tensor("hT_out", [H, BC], bf, kind="ExternalOutput").ap()
    cT_out = nc.dram_tensor("cT_out", [H, BC], bf, kind="ExternalOutput").ap()

    with tile.TileContext(nc) as tc:
        with tc.tile_pool(name="cst", bufs=1) as cst, \
             tc.tile_pool(name="xin", bufs=3) as xin, \
             tc.tile_pool(name="hin", bufs=3) as hin, \
             tc.tile_pool(name="cin", bufs=3) as cin, \
             tc.tile_pool(name="hout", bufs=2) as hout, \
             tc.tile_pool(name="cout", bufs=2) as cout, \
             tc.tile_pool(name="work", bufs=2) as work, \
             tc.tile_pool(name="ps", bufs=2, space="PSUM") as ps:

            W_sb = cst.tile([E, 4 * H], bf)
            U_sb = cst.tile([H, 4 * H], bf)
            b_sb = cst.tile([H, 4], f32)

            # first data + constants moving before anything else
            x0_sb = xin.tile([E, GROUP], bf, name="x0_sb", tag="x_sb")
            h0_sb = hin.tile([H, GROUP], bf, name="h0_sb", tag="h_sb")
            c0_sb = cin.tile([H, GROUP], bf, name="c0_sb", tag="c_sb")
            nc.sync.dma_start(out=x0_sb[:, 0:512], in_=xT[:, 0:512])
            nc.sync.dma_start(out=h0_sb[:, 0:512], in_=hT[:, 0:512])
            nc.sync.dma_start(out=W_sb[:], in_=W)
            nc.sync.dma_start(out=U_sb[:], in_=U)
            nc.sync.dma_start(out=b_sb[:], in_=bias)
            nc.sync.dma_start(out=c0_sb[:, 0:512], in_=cT[:, 0:512])

            # preload the sigmoid/tanh ACT table set while the first
            # group's DMA is in flight (the table swap costs ~2.7us)
            wsrc = cst.tile([E, 16], bf, name="wsrc")
            dmy = cst.tile([E, 16], bf, name="dmy")
            nc.vector.memset(wsrc[:], 1.0)
            nc.scalar.activation(dmy[:], wsrc[:], AF.Sigmoid)

            # variable-width groups: taper at both ends (earlier first
            # activation; shorter drain tail). Middle groups full width.
            widths = [512, 1536] + [GROUP] * (NG - 2) + [1536, 512]
            offs = [sum(widths[:i]) for i in range(len(widths))]

            def emit_tail(prev):
                """tanh(c) + h = o*tanh(c) + h DMA-out for a finished group.

                Deferred one group so the ACT-FIFO tanh never waits on the
                DVE chain of its own group (head-of-line blocking).
                """
                p_off, p_w, p_o, p_co = prev
                tc_sb = work.tile([H, p_w], bf, tag="tc")
                nc.scalar.activation(tc_sb[:], p_co[:, 0:p_w], AF.Tanh)
                ho_sb = hout.tile([H, p_w], bf, tag="ho")
                nc.vector.tensor_mul(out=ho_sb[:], in0=p_o[:, 0:p_w],
                                     in1=tc_sb[:])
                nparts = 2 if p_w <= 1024 else 1
                PC = p_w // nparts
                for hf in range(nparts):
                    o2, o3 = hf * PC, p_off + hf * PC
                    nc.sync.dma_start(out=hT_out[:, o3:o3 + PC],
                                      in_=ho_sb[:, o2:o2 + PC])

            prev = None
            for ch, (off, w) in enumerate(zip(offs, widths)):
                if ch == 0:
                    x_sb, h_sb, c_sb = x0_sb, h0_sb, c0_sb
                else:
                    x_sb = xin.tile([E, GROUP], bf, tag="x_sb")
                    h_sb = hin.tile([H, GROUP], bf, tag="h_sb")
                    c_sb = cin.tile([H, GROUP], bf, tag="c_sb")
                    for o2 in range(0, w, HALF):
                        pw = min(HALF, w - o2)
                        o3 = off + o2
                        nc.sync.dma_start(out=x_sb[:, o2:o2 + pw],
                                          in_=xT[:, o3:o3 + pw])
                        nc.sync.dma_start(out=h_sb[:, o2:o2 + pw],
                                          in_=hT[:, o3:o3 + pw])
                    nc.sync.dma_start(out=c_sb[:, 0:w], in_=cT[:, off:off + w])

                sig = {}
                for g in (GF, GC, GI, GO):
                    Wg = W_sb[:, g * H:(g + 1) * H]
                    Ug = U_sb[:, g * H:(g + 1) * H]
                    ps_g = ps.tile([H, GROUP], f32, tag="ps")
                    for q in range(w // 512):
                        qo = q * 512
                        nc.tensor.matmul(ps_g[:, qo:qo + 512], Wg,
                                         x_sb[:, qo:qo + 512],
                                         start=True, stop=False)
                    for q in range(w // 512):
                        qo = q * 512
                        nc.tensor.matmul(ps_g[:, qo:qo + 512], Ug,
                                         h_sb[:, qo:qo + 512],
                                         start=False, stop=True)
                    s_g = work.tile([H, GROUP], bf, tag=f"sig{g}", bufs=2)
                    fn = AF.Tanh if g == GC else AF.Sigmoid
                    nc.scalar.activation(s_g[:, 0:w], ps_g[:, 0:w], fn,
                                         bias=b_sb[:, g:g + 1])
                    sig[g] = s_g
                    if g == GC and prev is not None:
                        # slot the previous group's tanh(c)+h between this
                        # group's gate activations (its input is ready)
                        emit_tail(prev)
                        prev = None

                co_sb = cout.tile([H, GROUP], bf)
                m1 = work.tile([H, GROUP], bf, tag="m1")
                m2 = work.tile([H, GROUP], bf, tag="m2")
                nc.vector.tensor_mul(out=m1[:, 0:w], in0=sig[GF][:, 0:w],
                                     in1=c_sb[:, 0:w])
                nc.vector.tensor_mul(out=m2[:, 0:w], in0=sig[GI][:, 0:w],
                                     in1=sig[GC][:, 0:w])
                nc.vector.tensor_add(out=co_sb[:, 0:w], in0=m1[:, 0:w],
                                     in1=m2[:, 0:w])
                nc.sync.dma_start(out=cT_out[:, off:off + w],
                                  in_=co_sb[:, 0:w])
                prev = (off, w, sig[GO], co_sb)

            emit_tail(prev)

    nc.compile()
    return nc


def kernel(x, hidden_memory_tm1, Wi, Ui, bi, Wf, Uf, bf, Wog, Uog, bog,
           Wc, Uc, bc, _return_timing=False, _trace=False):
    from concourse.bass_utils import run_bass_kernel_spmd

    if "nc" not in _CACHE:
        _CACHE["nc"] = _build_nc()
    nc = _CACHE["nc"]

    import ml_dtypes
    bf16 = ml_dtypes.bfloat16
    x = np.asarray(x, np.float32)
    hm = np.asarray(hidden_memory_tm1, np.float32)
    # gate order f, c~, i, o (c~ uses Tanh directly on the ACT engine)
    W = np.concatenate([Wf, Wc, Wi, Wog], axis=1).astype(bf16)
    U = np.concatenate([Uf, Uc, Ui, Uog], axis=1).astype(bf16)
    b = np.stack([np.asarray(bf), np.asarray(bc),
                  np.asarray(bi), np.asarray(bog)], axis=1).astype(np.float32)
    b = np.ascontiguousarray(b)  # [H, 4], column g = per-partition bias

    in_maps = []
    for c in range(NCORES):
        sl = slice(c * BC, (c + 1) * BC)
        in_maps.append({
            "xT": np.ascontiguousarray(x[sl].astype(bf16).T),
            "hT": np.ascontiguousarray(hm[0, sl].astype(bf16).T),
            "cT": np.ascontiguousarray(hm[1, sl].astype(bf16).T),
            "W": W, "U": U, "b": b,
        })

    res = run_bass_kernel_spmd(nc, in_maps, core_ids=list(range(NCORES)),
                               trace=_trace)

    h = np.concatenate(
        [res.results[c]["hT_out"].T.astype(np.float32) for c in range(NCORES)], 0)
    cc = np.concatenate(
        [res.results[c]["cT_out"].T.astype(np.float32) for c in range(NCORES)], 0)
    out = np.stack([h, cc])
    if _return_timing:
        return out, res
    return out


# revision 13
# speedup vs baseline: 1.0154x; 1.0154x over previous
"""Single-step LSTM cell (B=131072, E=H=128) on 8 Trainium2 NeuronCores.

Strategy: pure data-parallel over the batch. Each core handles 16384 rows.
Host-side we pre-transpose each shard (x^T, h^T, c^T: [128, Bc] bf16) so the
contraction dim (E/H) lands on SBUF partitions — no on-chip transposes.

Per 2048-column group the four gate pre-activations are computed as
per-gate PSUM tiles [128, 2048] (4 banks each, 2-slot ring) with two
bf16 FD=1024 matmuls per stationary (W@x, U@h accumulate). The gate
bias enters through the ACT engine's per-partition bias operand on the
activation instruction itself — no bias matmuls at all. Each gate gets
one ACTIVATE (Sigmoid for i/f/o, Tanh for c~) at FD=2048, and tanh(c)
is one more ACTIVATE from SBUF. Elementwise (f*c_prev + i*c~, o*tanh(c))
runs on DVE fully in bf16 (2x mode). The c path moves as bf16 in both
directions, halving its HBM traffic and keeping every DVE op in 2x.

Steady state is ACT-bound: 5 ACTIVATEs x (2048+352)cyc / 1.2GHz = 10us
per group, 8 groups = 80us/core, with PE (~55us), DVE (~36us) and DMA
(20 MiB ~ 59us) all hidden underneath.
"""

import numpy as np

B, E, H = 131072, 128, 128
NCORES = 8
BC = B // NCORES        # 16384 batch rows per core
GROUP = 2048            # batch cols per pipeline group
HALF = GROUP // 2       # matmul moving-operand FD (bf16 max 1024)
NG = BC // GROUP

_CACHE = {}

# gate order in the weight/bias concatenation AND in per-group compute:
# f first (unblocks m1 = f*c_prev early), then c~, then i, then o.
GF, GC, GI, GO = 0, 1, 2, 3


def _build_nc():
    import concourse.bacc as bacc
    import concourse.mybir as mybir
    import concourse.tile as tile

    f32 = mybir.dt.float32
    bf = mybir.dt.bfloat16
    AF = mybir.ActivationFunctionType

    nc = bacc.Bacc("TRN2", target_bir_lowering=False, debug=False,
                   num_devices=NCORES)

    xT = nc.dram_tensor("xT", [E, BC], bf, kind="ExternalInput").ap()
    hT = nc.dram_tensor("hT", [H, BC], bf, kind="ExternalInput").ap()
    cT = nc.dram_tensor("cT", [H, BC], bf, kind="ExternalInput").ap()
    W = nc.dram_tensor("W", [E, 4 * H], bf, kind="ExternalInput").ap()
    U = nc.dram_tensor("U", [H, 4 * H], bf, kind="ExternalInput").ap()
    bias = nc.dram_tensor("b", [H, 4], f32, kind="ExternalInput").ap()
    hT_out = nc.dram_tensor("hT_out", [H, BC], bf, kind="ExternalOutput").ap()
    cT_out = nc.dram_tensor("cT_out", [H, BC], bf, kind="ExternalOutput").ap()

    with tile.TileContext(nc) as tc:
        with tc.tile_pool(name="cst", bufs=1) as cst, \
             tc.tile_pool(name="xin", bufs=3) as xin, \
             tc.tile_pool(name="hin", bufs=3) as hin, \
             tc.tile_pool(name="cin", bufs=3) as cin, \
             tc.tile_pool(name="hout", bufs=2) as hout, \
             tc.tile_pool(name="cout", bufs=2) as cout, \
             tc.tile_pool(name="work", bufs=2) as work, \
             tc.tile_pool(name="ps", bufs=2, space="PSUM") as ps:

            W_sb = cst.tile([E, 4 * H], bf)
            U_sb = cst.tile([H, 4 * H], bf)
            b_sb = cst.tile([H, 4], f32)

            # first data + constants moving before anything else
            x0_sb = xin.tile([E, GROUP], bf, name="x0_sb", tag="x_sb")
            h0_sb = hin.tile([H, GROUP], bf, name="h0_sb", tag="h_sb")
            c0_sb = cin.tile([H, GROUP], bf, name="c0_sb", tag="c_sb")
            nc.sync.dma_start(out=W_sb[:], in_=W)
            nc.sync.dma_start(out=x0_sb[:, 0:512], in_=xT[:, 0:512])
            nc.sync.dma_start(out=h0_sb[:, 0:512], in_=hT[:, 0:512])
            nc.sync.dma_start(out=U_sb[:], in_=U)
            nc.sync.dma_start(out=b_sb[:], in_=bias)
            nc.sync.dma_start(out=c0_sb[:, 0:512], in_=cT[:, 0:512])

            # preload the sigmoid/tanh ACT table set while the first
            # group's DMA is in flight (the table swap costs ~2.7us);
            # gated only on the W DMA so it fires as soon as the ACT
            # engine preamble finishes
            dmy = cst.tile([E, 16], bf, name="dmy")
            nc.scalar.activation(dmy[:], W_sb[:, 0:16], AF.Sigmoid)

            # variable-width groups: taper at both ends (earlier first
            # activation; shorter drain tail). Middle groups full width.
            widths = [512, 1536] + [GROUP] * (NG - 2) + [1536, 512]
            offs = [sum(widths[:i]) for i in range(len(widths))]

            def emit_tail(prev):
                """tanh(c) + h = o*tanh(c) + h DMA-out for a finished group.

                Deferred one group so the ACT-FIFO tanh never waits on the
                DVE chain of its own group (head-of-line blocking).
                """
                p_off, p_w, p_o, p_co = prev
                tc_sb = work.tile([H, p_w], bf, tag="tc")
                nc.scalar.activation(tc_sb[:], p_co[:, 0:p_w], AF.Tanh)
                ho_sb = hout.tile([H, p_w], bf, tag="ho")
                nc.vector.tensor_mul(out=ho_sb[:], in0=p_o[:, 0:p_w],
                                     in1=tc_sb[:])
                nparts = 2 if p_w <= 1024 else 1
                PC = p_w // nparts
                for hf in range(nparts):
                    o2, o3 = hf * PC, p_off + hf * PC
                    nc.sync.dma_start(out=hT_out[:, o3:o3 + PC],
                                      in_=ho_sb[:, o2:o2 + PC])

            prev = None
            for ch, (off, w) in enumerate(zip(offs, widths)):
                if ch == 0:
                    x_sb, h_sb, c_sb = x0_sb, h0_sb, c0_sb
                else:
                    x_sb = xin.tile([E, GROUP], bf, tag="x_sb")
                    h_sb = hin.tile([H, GROUP], bf, tag="h_sb")
                    c_sb = cin.tile([H, GROUP], bf, tag="c_sb")
                    for o2 in range(0, w, HALF):
                        pw = min(HALF, w - o2)
                        o3 = off + o2
                        nc.sync.dma_start(out=x_sb[:, o2:o2 + pw],
                                          in_=xT[:, o3:o3 + pw])
                        nc.sync.dma_start(out=h_sb[:, o2:o2 + pw],
                                          in_=hT[:, o3:o3 + pw])
                    nc.sync.dma_start(out=c_sb[:, 0:w], in_=cT[:, off:off + w])

                sig = {}
                for g in (GF, GC, GI, GO):
                    Wg = W_sb[:, g * H:(g + 1) * H]
                    Ug = U_sb[:, g * H:(g + 1) * H]
                    ps_g = ps.tile([H, GROUP], f32, tag="ps")
                    for q in range(w // 512):
                        qo = q * 512
                        nc.tensor.matmul(ps_g[:, qo:qo + 512], Wg,
                                         x_sb[:, qo:qo + 512],
                                         start=True, stop=False)
                    for q in range(w // 512):
                        qo = q * 512
                        nc.tensor.matmul(ps_g[:, qo:qo + 512], Ug,
                                         h_sb[:, qo:qo + 512],
                                         start=False, stop=True)
                    s_g = work.tile([H, GROUP], bf, tag=f"sig{g}", bufs=2)
                    fn = AF.Tanh if g == GC else AF.Sigmoid
                    nc.scalar.activation(s_g[:, 0:w], ps_g[:, 0:w], fn,
                                         bias=b_sb[:, g:g + 1])
                    sig[g] = s_g
                    if g == GC and prev is not None:
                        # slot the previous group's tanh(c)+h between this
                        # group's gate activations (its input is ready)
                        emit_tail(prev)
                        prev = None

                co_sb = cout.tile([H, GROUP], bf)
                m1 = work.tile([H, GROUP], bf, tag="m1")
                m2 = work.tile([H, GROUP], bf, tag="m2")
                nc.vector.tensor_mul(out=m1[:, 0:w], in0=sig[GF][:, 0:w],
                                     in1=c_sb[:, 0:w])
                nc.vector.tensor_mul(out=m2[:, 0:w], in0=sig[GI][:, 0:w],
                                     in1=sig[GC][:, 0:w])
                nc.vector.tensor_add(out=co_sb[:, 0:w], in0=m1[:, 0:w],
                                     in1=m2[:, 0:w])
                nc.sync.dma_start(out=cT_out[:, off:off + w],
                                  in_=co_sb[:, 0:w])
                prev = (off, w, sig[GO], co_sb)

            emit_tail(prev)

    nc.compile()
    return nc


def kernel(x, hidden_memory_tm1, Wi, Ui, bi, Wf, Uf, bf, Wog, Uog, bog,
           Wc, Uc, bc, _return_timing=False, _trace=False):
    from concourse.bass_utils import run_bass_kernel_spmd

    if "nc" not in _CACHE:
        _CACHE["nc"] = _build_nc()
    nc = _CACHE["nc"]

    import ml_dtypes
    bf16 = ml_dtypes.bfloat16
    x = np.asarray(x, np.float32)
    hm = np.asarray(hidden_memory_tm1, np.float32)
    # gate order f, c~, i, o (c~ uses Tanh directly on the ACT engine)
    W = np.concatenate([Wf, Wc, Wi, Wog], axis=1).astype(bf16)
    U = np.concatenate([Uf, Uc, Ui, Uog], axis=1).astype(bf16)
    b = np.stack([np.asarray(bf), np.asarray(bc),
                  np.asarray(bi), np.asarray(bog)], axis=1).astype(np.float32)
    b = np.ascontiguousarray(b)  # [H, 4], column g = per-partition bias

    in_maps = []
    for c in range(NCORES):
        sl = slice(c * BC, (c + 1) * BC)
        in_maps.append({
            "xT": np.ascontiguousarray(x[sl].astype(bf16).T),
            "hT": np.ascontiguousarray(hm[0, sl].astype(bf16).T),
            "cT": np.ascontiguousarray(hm[1, sl].astype(bf16).T),
            "W": W, "U": U, "b": b,
        })

    res = run_bass_kernel_spmd(nc, in_maps, core_ids=list(range(NCORES)),
                               trace=_trace)

    h = np.concatenate(
        [res.results[c]["hT_out"].T.astype(np.float32) for c in range(NCORES)], 0)
    cc = np.concatenate(
        [res.results[c]["cT_out"].T.astype(np.float32) for c in range(NCORES)], 0)
    out = np.stack([h, cc])
    if _return_timing:
        return out, res
    return out
